# revision 1
# baseline (speedup 1.0000x reference)
"""Trainium2 Bass kernel for AttnApply (sliding-window weighted sum).

out[b, t, c] = sum_i padded[b, t+i, c] * weights[b, t, i]   (T=11, D=5 zero pad)

Strategy
--------
Pure data parallel over batch: 8 cores x 4 batches each.

Per core, the windowed sum is a banded matrix multiply on the TensorEngine.
For a time block of M=118 output rows starting at t0 (K = M+T-1 = 128):

    out[t0+m, c] = sum_k band[k, m] * in_pad[t0+k, c],   k in [0, 128)

with band[k, m] = w[t0+m, k-m] for 0 <= k-m < T (zero elsewhere); in_pad is
host zero-padded so edge blocks need no special casing.  Band matrices are
built host-side (cheap scatter of the small weights tensor).

The matmul runs with the INPUT tile as the stationary operand and the band as
the moving operand, producing the TRANSPOSED output in PSUM:

    psum[c, m] = sum_k in_pad[t0+k, c] * band[k, m]

so PSUM partitions are channels (two 128-channel halves) and the free dim is
time.  Channel-major output means each partition's store is a long contiguous
run in a [C, L] DRAM tensor (host un-transposes at the end) — measured ~5x
faster than time-major stores, which degrade to 1KB-per-descriptor writes
(~60 GB/s vs ~310+ GB/s on this part).

Precision/speed: operands are host-split into bf16 hi+lo pairs (packed along
the last axis, same total bytes as fp32) and each product is computed as
ih@bh + ih@bl + il@bh accumulated in fp32 PSUM — 3 bf16 matmul passes
(1 cyc/row each) instead of fp32's 4 cyc/row, with ~2^-18 relative error
(measured 4e-6 end-to-end vs the fp32 reference; plain bf16 would be ~1e-3).

Layout per supertile of J=7 blocks:
 - 7 per-block input loads [128, 2C] bf16 (contiguous, SP queue)
 - 1 band load [128, 2*J*M] bf16 (contiguous, SP/ACT alternating)
 - 42 matmuls (7 blocks x 2 channel halves x 3 split passes) into psum
   [128, J*128] (block stride padded 118->128 so every matmul output is
   bank-aligned)
 - compact psum -> SBUF copies split across VectorE and ScalarE
 - 2 column-major stores [128, 826] per supertile on ACT's HWDGE queue
"""

import ml_dtypes
import numpy as np

import concourse.bass as bass  # noqa: F401  (engine handles hang off nc)
import concourse.mybir as mybir
import concourse.tile as tile
from concourse import bacc
from concourse.bass_utils import run_bass_kernel_spmd

B, L, C, T = 32, 4096, 256, 11
D = T // 2
N_CORES = 8
B_LOC = B // N_CORES            # 4 batches per core
M = 118                         # output rows per matmul block
K = M + T - 1                   # 128 = contraction rows per block
NBLK = -(-L // M)               # 35 blocks per batch
J = 7                           # blocks per supertile
NSUP = NBLK // J                # 5 supertiles per batch
SUP = M * J                     # 826 output rows per supertile
MP = 128                        # padded per-block psum stride (bank aligned)
LPAD = (NBLK - 1) * M + K       # 4140 padded input rows

_CACHE: dict = {}
LAST_RESULT = None  # BassKernelResults of the most recent run (for test.py)


def _build_nc(repeat: int = 1, bench: bool = False):
    """Build the bass program. `repeat` re-runs the whole body N times and
    `bench=True` uses internal zero-filled DRAM inputs/outputs with only a
    tiny external "tick" output — both used only for benchmarking; the
    grading path uses repeat=1, bench=False."""
    nc = bacc.Bacc(
        "TRN2",
        target_bir_lowering=False,
        debug=False,
        num_devices=N_CORES,
    )
    if bench:
        inp = nc.dram_tensor(
            "in_int", [B_LOC, LPAD, 2 * C], mybir.dt.bfloat16
        ).ap()
        band = nc.dram_tensor(
            "band_int", [B_LOC, NSUP, K, 2 * J * M], mybir.dt.bfloat16
        ).ap()
        outT = nc.dram_tensor("outT_int", [B_LOC, C, L], mybir.dt.float32).ap()
        tick = nc.dram_tensor(
            "tick", [1, C], mybir.dt.float32, kind="ExternalOutput"
        ).ap()
    else:
        inp = nc.dram_tensor(
            "in_pad", [B_LOC, LPAD, 2 * C], mybir.dt.bfloat16, kind="ExternalInput"
        ).ap()
        band = nc.dram_tensor(
            "band",
            [B_LOC, NSUP, K, 2 * J * M],
            mybir.dt.bfloat16,
            kind="ExternalInput",
        ).ap()
        outT = nc.dram_tensor(
            "outT", [B_LOC, C, L], mybir.dt.float32, kind="ExternalOutput"
        ).ap()
        tick = None

    with tile.TileContext(nc) as tc:
        with (
            tc.tile_pool(name="inp", bufs=10) as in_pool,
            tc.tile_pool(name="bnd", bufs=4) as bd_pool,
            tc.tile_pool(name="outp", bufs=4) as o_pool,
            tc.tile_pool(name="ps", bufs=4, space="PSUM") as ps_pool,
        ):
            if bench:
                # back every DRAM page with zeros once per run so reads are
                # real HBM traffic (unbacked-page reads measure absurdly
                # fast and would not represent the grading path)
                with tc.tile_pool(name="z", bufs=1) as z_pool:
                    z = z_pool.tile([K, SUP], mybir.dt.float32, tag="z")
                    nc.gpsimd.memset(z[:, :], 0.0)
                    for b in range(B_LOC):
                        for r0 in range(0, LPAD, K):
                            cnt = min(K, LPAD - r0)
                            nc.sync.dma_start(
                                out=inp[b, r0 : r0 + cnt, :],
                                in_=z[:cnt, :C].bitcast(mybir.dt.bfloat16),
                            )
                        for s in range(NSUP):
                            nc.sync.dma_start(
                                out=band[b, s],
                                in_=z[:, : J * M].bitcast(mybir.dt.bfloat16),
                            )
                        for ch in range(2):
                            for s in range(NSUP):
                                lo, hi = s * SUP, min((s + 1) * SUP, L)
                                nc.sync.dma_start(
                                    out=outT[b, ch * 128 : (ch + 1) * 128, lo:hi],
                                    in_=z[:, : hi - lo],
                                )

            for _rep in range(repeat):
                for b in range(B_LOC):
                    for s in range(NSUP):
                        t0 = s * SUP
                        # ---- band load (alternate SP/ACT queues so both
                        # carry ~22MB: SP gets the input loads, ACT the
                        # stores, band tops up whichever has less) ----
                        bd_t = bd_pool.tile(
                            [K, 2 * J * M], mybir.dt.bfloat16, tag="bd"
                        )
                        beng = nc.sync if s % 2 == 0 else nc.scalar
                        beng.dma_start(out=bd_t[:, :], in_=band[b, s])

                        # ---- per-block input loads (SP HWDGE queue) ----
                        in_ts = []
                        for jj in range(J):
                            tb = t0 + jj * M
                            in_t = in_pool.tile(
                                [K, 2 * C], mybir.dt.bfloat16, tag="in"
                            )
                            nc.sync.dma_start(
                                out=in_t[:, :], in_=inp[b, tb : tb + K, :]
                            )
                            in_ts.append(in_t)

                        # ---- matmuls: psum[c, m] per channel half ----
                        pss = []
                        for ch in range(2):
                            ps = ps_pool.tile(
                                [128, J * MP], mybir.dt.float32, tag="ps"
                            )
                            for jj in range(J):
                                ih = in_ts[jj][:, ch * 128 : (ch + 1) * 128]
                                il = in_ts[jj][:, C + ch * 128 : C + (ch + 1) * 128]
                                bh = bd_t[:, jj * M : (jj + 1) * M]
                                bl = bd_t[:, J * M + jj * M : J * M + (jj + 1) * M]
                                out_sl = ps[:, jj * MP : jj * MP + M]
                                nc.tensor.matmul(out_sl, ih, bh, start=True, stop=False)
                                nc.tensor.matmul(out_sl, ih, bl, start=False, stop=False)
                                nc.tensor.matmul(out_sl, il, bh, start=False, stop=True)
                            pss.append(ps)

                        # ---- compact copy (DVE + ACT) + column-major store
                        # (both stores on ACT's HWDGE queue, keeping SP's
                        # queue dedicated to input loads) ----
                        rows = min(SUP, L - t0)
                        for ch in range(2):
                            o_t = o_pool.tile([128, SUP], mybir.dt.float32, tag="o")
                            src = pss[ch].rearrange("p (j m) -> p j m", j=J)[
                                :, :, :M
                            ]
                            dst = o_t[:, :].rearrange("p (j m) -> p j m", j=J)
                            if ch == 0:
                                nc.vector.tensor_copy(out=dst, in_=src)
                            else:
                                nc.scalar.copy(out=dst, in_=src)
                            nc.scalar.dma_start(
                                out=outT[b, ch * 128 : (ch + 1) * 128, t0 : t0 + rows],
                                in_=o_t[:, :rows],
                            )
                if tick is not None:
                    # flush both HWDGE queues: same-queue reads complete only
                    # after all prior writes on that queue
                    fl = o_pool.tile([2, C], mybir.dt.float32, tag="fl")
                    nc.sync.dma_start(out=fl[0:1, :], in_=outT[0, 0:1, 0:C])
                    nc.scalar.dma_start(out=fl[1:2, :], in_=outT[0, 128:129, 0:C])
                    nc.sync.dma_start(out=tick[:, :], in_=fl[0:1, :])
                    nc.sync.dma_start(out=tick[:, 0:C], in_=fl[1:2, :])
    nc.compile()
    return nc


BF16 = ml_dtypes.bfloat16


def _split_hi_lo(x: np.ndarray):
    """fp32 -> (hi, lo) bf16 pair with x ~= hi + lo (error ~2^-18 rel)."""
    hi = x.astype(BF16)
    lo = (x - hi.astype(np.float32)).astype(BF16)
    return hi, lo


def _prep_core(x: np.ndarray, w: np.ndarray):
    """x: [B_LOC, L, C] f32, w: [B_LOC, L, T] f32 -> (in_pad, band),
    each with bf16 hi/lo halves packed along the last axis."""
    in_f32 = np.zeros((B_LOC, LPAD, C), np.float32)
    in_f32[:, D : D + L, :] = x
    in_pad = np.empty((B_LOC, LPAD, 2 * C), BF16)
    in_pad[:, :, :C], in_pad[:, :, C:] = _split_hi_lo(in_f32)
    band_f32 = np.zeros((B_LOC, NBLK, K, M), np.float32)
    jj, mm = np.meshgrid(np.arange(NBLK), np.arange(M), indexing="ij")
    tt = jj * M + mm
    v = tt < L
    jv, mv_, tv = jj[v], mm[v], tt[v]
    for tau in range(T):
        band_f32[:, jv, mv_ + tau, mv_] = w[:, tv, tau]
    # regroup into supertile layout [B_LOC, NSUP, K, J*M]
    band_f32 = np.ascontiguousarray(
        band_f32.reshape(B_LOC, NSUP, J, K, M).transpose(0, 1, 3, 2, 4)
    ).reshape(B_LOC, NSUP, K, J * M)
    band = np.empty((B_LOC, NSUP, K, 2 * J * M), BF16)
    band[..., : J * M], band[..., J * M :] = _split_hi_lo(band_f32)
    return in_pad, band


def kernel(inputs: np.ndarray, weights: np.ndarray) -> np.ndarray:
    global LAST_RESULT
    inputs = np.ascontiguousarray(np.asarray(inputs, dtype=np.float32))
    weights = np.ascontiguousarray(np.asarray(weights, dtype=np.float32))
    assert inputs.shape == (B, L, C) and weights.shape == (B, L, T)

    if "nc" not in _CACHE:
        _CACHE["nc"] = _build_nc()
    nc = _CACHE["nc"]

    in_maps = []
    for c in range(N_CORES):
        sl = slice(c * B_LOC, (c + 1) * B_LOC)
        ip, bd = _prep_core(inputs[sl], weights[sl])
        in_maps.append({"in_pad": ip, "band": bd})

    res = run_bass_kernel_spmd(nc, in_maps, core_ids=list(range(N_CORES)))
    LAST_RESULT = res
    # outputs come back channel-major [B_LOC, C, L]; un-transpose on host
    return np.ascontiguousarray(
        np.concatenate(
            [r["outT"].transpose(0, 2, 1) for r in res.results], axis=0
        )
    )



# revision 3
# speedup vs baseline: 4.2551x; 4.2551x over previous
"""Trainium2 Bass kernel for AttnApply (sliding-window weighted sum).

out[b, t, c] = sum_i padded[b, t+i, c] * weights[b, t, i]   (T=11, D=5 zero pad)

Strategy
--------
Pure data parallel over batch: 8 cores x 4 batches each.

Per core, the windowed sum is a banded matrix multiply on the TensorEngine.
For a time block of M=118 output rows starting at t0 (K = M+T-1 = 128):

    out[t0+m, c] = sum_k band[k, m] * in_pad[t0+k, c],   k in [0, 128)

with band[k, m] = w[t0+m, k-m] for 0 <= k-m < T (zero elsewhere); in_pad is
host zero-padded so edge blocks need no special casing.  Band matrices are
built host-side (cheap scatter of the small weights tensor).

The matmul runs with the INPUT tile as the stationary operand and the band as
the moving operand, producing the TRANSPOSED output in PSUM:

    psum[c, m] = sum_k in_pad[t0+k, c] * band[k, m]

so PSUM partitions are channels (two 128-channel halves) and the free dim is
time.  Channel-major output means each partition's store is a long contiguous
run in a [C, L] DRAM tensor (host un-transposes at the end).

Precision/speed: the correctness gate is rel_err < 2e-2, so everything runs
in plain bf16 (measured ~3e-3 end-to-end): bf16 inputs, bf16 band, single
1-cyc/row matmul pass per (block, channel-half), fp32 PSUM accumulation, and
bf16 stores (host converts back to f32).  Relative to the fp32-accurate hi/lo
split this halves every DMA stream and cuts matmul passes 3x.

Layout per core:
 - per block: input load [128, C] bf16 (contiguous, SP queue)
 - per supertile of J=7 blocks: 1 band load [128, J*M] bf16 (alternating
   SP/ACT queues to balance bytes)
 - 14 matmuls per supertile (7 blocks x 2 channel halves) into psum
   [128, J*128] (block stride padded 118->128 so every matmul output is
   bank-aligned)
 - compact psum -> batch-wide SBUF tile [128, 4130] bf16 (f32->bf16 on
   VectorE for half 0, ScalarE for half 1)
 - per batch per half: ONE [128, 4096] bf16 store (8KB contiguous per
   partition, whole region contiguous in DRAM) on ACT's HWDGE queue
"""

import ml_dtypes
import numpy as np

import concourse.bass as bass  # noqa: F401  (engine handles hang off nc)
import concourse.mybir as mybir
import concourse.tile as tile
from concourse import bacc
from concourse.bass_utils import run_bass_kernel_spmd

B, L, C, T = 32, 4096, 256, 11
D = T // 2
N_CORES = 8
B_LOC = B // N_CORES            # 4 batches per core
M = 118                         # output rows per matmul block
K = M + T - 1                   # 128 = contraction rows per block
NBLK = -(-L // M)               # 35 blocks per batch
J = 7                           # blocks per supertile
NSUP = NBLK // J                # 5 supertiles per batch
SUP = M * J                     # 826 output rows per supertile
MP = 128                        # padded per-block psum stride (bank aligned)
LPAD = (NBLK - 1) * M + K       # 4140 padded input rows
LACC = NBLK * M                 # 4130 accumulated output rows per batch

_CACHE: dict = {}
LAST_RESULT = None  # BassKernelResults of the most recent run (for test.py)


def _build_nc(repeat: int = 1, bench: bool = False):
    """Build the bass program. `repeat` re-runs the whole body N times and
    `bench=True` uses internal zero-filled DRAM inputs/outputs with only a
    tiny external "tick" output — both used only for benchmarking; the
    grading path uses repeat=1, bench=False."""
    nc = bacc.Bacc(
        "TRN2",
        target_bir_lowering=False,
        debug=False,
        num_devices=N_CORES,
    )
    if bench:
        inp = nc.dram_tensor(
            "in_int", [B_LOC, LPAD, C], mybir.dt.bfloat16
        ).ap()
        band = nc.dram_tensor(
            "band_int", [B_LOC, NSUP, K, J * M], mybir.dt.bfloat16
        ).ap()
        outT = nc.dram_tensor("outT_int", [B_LOC, C, L], mybir.dt.bfloat16).ap()
        tick = nc.dram_tensor(
            "tick", [1, C], mybir.dt.bfloat16, kind="ExternalOutput"
        ).ap()
    else:
        inp = nc.dram_tensor(
            "in_pad", [B_LOC, LPAD, C], mybir.dt.bfloat16, kind="ExternalInput"
        ).ap()
        band = nc.dram_tensor(
            "band",
            [B_LOC, NSUP, K, J * M],
            mybir.dt.bfloat16,
            kind="ExternalInput",
        ).ap()
        outT = nc.dram_tensor(
            "outT", [B_LOC, C, L], mybir.dt.bfloat16, kind="ExternalOutput"
        ).ap()
        tick = None

    with tile.TileContext(nc) as tc:
        with (
            tc.tile_pool(name="inp", bufs=10) as in_pool,
            tc.tile_pool(name="bnd", bufs=4) as bd_pool,
            tc.tile_pool(name="outp", bufs=4) as o_pool,
            tc.tile_pool(name="ps", bufs=4, space="PSUM") as ps_pool,
        ):
            if bench:
                # back every DRAM page with zeros once per run so reads are
                # real HBM traffic (unbacked-page reads measure absurdly
                # fast and would not represent the grading path)
                with tc.tile_pool(name="z", bufs=1) as z_pool:
                    z = z_pool.tile([K, SUP], mybir.dt.float32, tag="z")
                    nc.gpsimd.memset(z[:, :], 0.0)
                    for b in range(B_LOC):
                        for r0 in range(0, LPAD, K):
                            cnt = min(K, LPAD - r0)
                            nc.sync.dma_start(
                                out=inp[b, r0 : r0 + cnt, :],
                                in_=z[:cnt, : C // 2].bitcast(mybir.dt.bfloat16),
                            )
                        for s in range(NSUP):
                            nc.sync.dma_start(
                                out=band[b, s],
                                in_=z[:, : (J * M) // 2].bitcast(
                                    mybir.dt.bfloat16
                                ),
                            )
                        for ch in range(2):
                            for s in range(NSUP):
                                lo, hi = s * SUP, min((s + 1) * SUP, L)
                                nc.sync.dma_start(
                                    out=outT[b, ch * 128 : (ch + 1) * 128, lo:hi],
                                    in_=z[:, : (hi - lo) // 2].bitcast(
                                        mybir.dt.bfloat16
                                    ),
                                )

            for _rep in range(repeat):
                for b in range(B_LOC):
                    # batch-wide output accumulators (one per channel half)
                    o_ts = [
                        o_pool.tile(
                            [128, LACC],
                            mybir.dt.bfloat16,
                            tag=f"o{ch}",
                            name=f"o_t{ch}",
                        )
                        for ch in range(2)
                    ]
                    for s in range(NSUP):
                        t0 = s * SUP
                        # ---- band load (alternate SP/ACT queues) ----
                        bd_t = bd_pool.tile(
                            [K, J * M], mybir.dt.bfloat16, tag="bd"
                        )
                        beng = nc.sync if s % 2 == 0 else nc.scalar
                        beng.dma_start(out=bd_t[:, :], in_=band[b, s])

                        # ---- per-block input loads (SP HWDGE queue) ----
                        in_ts = []
                        for jj in range(J):
                            tb = t0 + jj * M
                            in_t = in_pool.tile(
                                [K, C], mybir.dt.bfloat16, tag="in"
                            )
                            nc.sync.dma_start(
                                out=in_t[:, :], in_=inp[b, tb : tb + K, :]
                            )
                            in_ts.append(in_t)

                        # ---- matmuls: psum[c, m] per channel half ----
                        for ch in range(2):
                            ps = ps_pool.tile(
                                [128, J * MP], mybir.dt.float32, tag="ps"
                            )
                            for jj in range(J):
                                ih = in_ts[jj][:, ch * 128 : (ch + 1) * 128]
                                bh = bd_t[:, jj * M : (jj + 1) * M]
                                out_sl = ps[:, jj * MP : jj * MP + M]
                                nc.tensor.matmul(
                                    out_sl, ih, bh, start=True, stop=True
                                )
                            # ---- compact f32->bf16 copy into the batch
                            # accumulator (DVE for half 0, ACT for half 1) --
                            src = ps.rearrange("p (j m) -> p j m", j=J)[:, :, :M]
                            dst = o_ts[ch][:, t0 : t0 + SUP].rearrange(
                                "p (j m) -> p j m", j=J
                            )
                            if ch == 0:
                                nc.vector.tensor_copy(out=dst, in_=src)
                            else:
                                nc.scalar.copy(out=dst, in_=src)

                    # ---- one big contiguous store per channel half ----
                    for ch in range(2):
                        nc.scalar.dma_start(
                            out=outT[b, ch * 128 : (ch + 1) * 128, :],
                            in_=o_ts[ch][:, :L],
                        )
                if tick is not None:
                    # flush both HWDGE queues: same-queue reads complete only
                    # after all prior writes on that queue
                    fl = o_pool.tile([2, C], mybir.dt.bfloat16, tag="fl")
                    nc.sync.dma_start(out=fl[0:1, :], in_=outT[0, 0:1, 0:C])
                    nc.scalar.dma_start(out=fl[1:2, :], in_=outT[0, 128:129, 0:C])
                    nc.sync.dma_start(out=tick[:, :], in_=fl[0:1, :])
                    nc.sync.dma_start(out=tick[:, 0:C], in_=fl[1:2, :])
    nc.compile()
    return nc


BF16 = ml_dtypes.bfloat16


def _prep_core(x: np.ndarray, w: np.ndarray):
    """x: [B_LOC, L, C] f32, w: [B_LOC, L, T] f32 -> (in_pad, band), bf16."""
    in_pad = np.zeros((B_LOC, LPAD, C), BF16)
    in_pad[:, D : D + L, :] = x.astype(BF16)
    band16 = np.zeros((B_LOC, NBLK, K, M), BF16)
    jj, mm = np.meshgrid(np.arange(NBLK), np.arange(M), indexing="ij")
    tt = jj * M + mm
    v = tt < L
    jv, mv_, tv = jj[v], mm[v], tt[v]
    w16 = w.astype(BF16)
    for tau in range(T):
        band16[:, jv, mv_ + tau, mv_] = w16[:, tv, tau]
    # regroup into supertile layout [B_LOC, NSUP, K, J*M]
    band16 = np.ascontiguousarray(
        band16.reshape(B_LOC, NSUP, J, K, M).transpose(0, 1, 3, 2, 4)
    ).reshape(B_LOC, NSUP, K, J * M)
    return in_pad, band16


def kernel(inputs: np.ndarray, weights: np.ndarray) -> np.ndarray:
    global LAST_RESULT
    inputs = np.ascontiguousarray(np.asarray(inputs, dtype=np.float32))
    weights = np.ascontiguousarray(np.asarray(weights, dtype=np.float32))
    assert inputs.shape == (B, L, C) and weights.shape == (B, L, T)

    if "nc" not in _CACHE:
        _CACHE["nc"] = _build_nc()
    nc = _CACHE["nc"]

    in_maps = []
    for c in range(N_CORES):
        sl = slice(c * B_LOC, (c + 1) * B_LOC)
        ip, bd = _prep_core(inputs[sl], weights[sl])
        in_maps.append({"in_pad": ip, "band": bd})

    res = run_bass_kernel_spmd(nc, in_maps, core_ids=list(range(N_CORES)))
    LAST_RESULT = res
    # outputs come back channel-major bf16 [B_LOC, C, L]; un-transpose and
    # widen to f32 on host
    return np.ascontiguousarray(
        np.concatenate(
            [
                r["outT"].astype(np.float32).transpose(0, 2, 1)
                for r in res.results
            ],
            axis=0,
        )
    )
